# revision 1
# baseline (speedup 1.0000x reference)
"""BRF (bursting resonate-and-fire) neuron update kernel for Trainium2.

Computes, elementwise over [B=4096, D=4096] fp32 tensors (per-neuron
vectors omegas/bs/threshold along D):

    omega  = |omegas|
    p      = (-1 + sqrt(1 - (DT*omega)^2)) / DT
    b      = p - |bs| - q
    u_     = u + b*u*DT - omega*v*DT + x*DT
    v_new  = v + omega*u*DT + b*v*DT
    z      = heaviside(|u_| - |threshold| - q)
    q_new  = q*0.9 + z

Sharding: batch rows split evenly across 8 NeuronCores (data parallel,
contiguous row slabs -> zero-copy numpy views). Per-neuron [D] vectors are
folded host-side (O(D) work) into three constants

    C  = 1 + DT*(p - |bs|)     so that  u_ = A*u - W*v + DT*x
    W  = DT*omega                       v_ = A*v + W*u      with A = C - DT*q
    TH = |threshold|                    z  = (|u_| > TH + q)

and broadcast on-device to all 128 partitions.
"""

import os

import numpy as np

DT = 1.0 / 24000.0
Q_DECAY = 0.9
B, D = 4096, 4096
N_CORES = 8
ROWS = B // N_CORES  # rows per core
P = 128  # SBUF partitions

# Set by kernel() after a traced run (BRF_TRACE=1): ns of the slowest core.
LAST_EXEC_TIME_NS = None
LAST_RESULTS = None


def _legalize_bir_waits(raw: bytes) -> bytes:
    """Split multi-wait instructions into EventSemaphore + 1-wait instruction.

    The walrus build in this toolchain encodes at most ONE sync-wait per
    instruction; Tile's semaphore assignment emits several. Hoisting the
    extra waits onto standalone EventSemaphore instructions immediately
    before the instruction (same engine stream, in-order) is semantically
    identical.
    """
    import json

    d = json.loads(raw)
    n_split = 0
    for fn in d.get("functions", []):
        for bb in fn.get("blocks", []):
            out = []
            for ins in bb.get("instructions", []):
                si = ins.get("sync_info") or {}
                waits = si.get("on_wait") or []
                if len(waits) > 1:
                    for k, w in enumerate(waits[:-1]):
                        out.append(
                            {
                                "debug": ins.get("debug", 0),
                                "engine": ins["engine"],
                                "ins": [],
                                "name": f"{ins['name']}-w{k}",
                                "opcode": "EventSemaphore",
                                "outs": [],
                                "sync_info": {"on_update": [], "on_wait": [w]},
                            }
                        )
                        n_split += 1
                    si["on_wait"] = [waits[-1]]
                out.append(ins)
            bb["instructions"] = out
    return json.dumps(d).encode()


def _install_wait_legalizer(nc):
    orig = nc.to_json_bytes

    def patched():
        return _legalize_bir_waits(orig())

    nc.to_json_bytes = patched
    return nc


def build_nc(rows=ROWS, d=D, free=1024, repeat=1, dma_only=False,
             bcast_engine="gpsimd", inplace=True, z8=False, store_engine="sync",
             a_psum=False, io_bufs=None, tmp_bufs=None):
    """Build the per-core Bass program (identical on all 8 cores).

    repeat > 1 re-emits the whole main loop that many times (same work,
    same DRAM traffic each pass) — used only for slope-based timing.
    dma_only skips all compute and stores the loaded tiles straight back
    (same DMA traffic) — used to measure the pure memory floor.
    """
    import concourse.bass as bass
    import concourse.mybir as mybir
    from concourse.tile import TileContext

    f32 = mybir.dt.float32
    u8 = mybir.dt.uint8
    Alu = mybir.AluOpType
    Act = mybir.ActivationFunctionType

    nc = bass.Bass(trn_type="TRN2")

    x = nc.dram_tensor("x", [rows, d], f32, kind="ExternalInput")
    u = nc.dram_tensor("u", [rows, d], f32, kind="ExternalInput")
    v = nc.dram_tensor("v", [rows, d], f32, kind="ExternalInput")
    q = nc.dram_tensor("q", [rows, d], f32, kind="ExternalInput")
    cvec = nc.dram_tensor("cvec", [1, d], f32, kind="ExternalInput")
    wvec = nc.dram_tensor("wvec", [1, d], f32, kind="ExternalInput")
    tvec = nc.dram_tensor("tvec", [1, d], f32, kind="ExternalInput")

    z_o = nc.dram_tensor("z_o", [rows, d], u8 if z8 else f32, kind="ExternalOutput")
    u_o = nc.dram_tensor("u_o", [rows, d], f32, kind="ExternalOutput")
    v_o = nc.dram_tensor("v_o", [rows, d], f32, kind="ExternalOutput")
    q_o = nc.dram_tensor("q_o", [rows, d], f32, kind="ExternalOutput")

    n_pb = rows // P
    n_fc = d // free

    with TileContext(nc) as tc:
        if io_bufs is None:
            io_bufs = 4 if inplace else 3
        if tmp_bufs is None:
            tmp_bufs = 3 if inplace else 2
        with (
            tc.tile_pool(name="consts", bufs=1) as cp,
            tc.tile_pool(name="io", bufs=io_bufs) as iop,
            tc.tile_pool(name="tmp", bufs=tmp_bufs) as tp,
            tc.tile_pool(name="ps", bufs=2, space="PSUM") as pp,
        ):
            # Broadcast the three per-neuron vectors to all 128 partitions,
            # chunked so the first compute tiles aren't gated on full-width
            # broadcasts.
            Ct = cp.tile([P, d], f32, tag="C")
            Wt = cp.tile([P, d], f32, tag="W")
            Tt = cp.tile([P, d], f32, tag="T")
            bcast_dma = nc.gpsimd if bcast_engine == "gpsimd" else nc.sync
            for tile, handle in ((Ct, cvec), (Wt, wvec), (Tt, tvec)):
                src = handle[:]
                bc = bass.AP(tensor=src.tensor, offset=src.offset, ap=[[0, P], [1, d]])
                bcast_dma.dma_start(out=tile[:], in_=bc)

            for pb in range(n_pb * repeat):
                r0 = (pb % n_pb) * P
                for fc in range(n_fc):
                    c0 = fc * free
                    rs = slice(r0, r0 + P)
                    cs = slice(c0, c0 + free)

                    xt = iop.tile([P, free], f32, tag="x")
                    ut = iop.tile([P, free], f32, tag="u")
                    vt = iop.tile([P, free], f32, tag="v")
                    qt = iop.tile([P, free], f32, tag="q")
                    nc.sync.dma_start(out=xt[:], in_=x[rs, cs])
                    nc.sync.dma_start(out=ut[:], in_=u[rs, cs])
                    nc.sync.dma_start(out=vt[:], in_=v[rs, cs])
                    nc.sync.dma_start(out=qt[:], in_=q[rs, cs])

                    st = nc.sync if store_engine == "sync" else nc.scalar
                    if dma_only:
                        st.dma_start(out=u_o[rs, cs], in_=xt[:])
                        st.dma_start(out=v_o[rs, cs], in_=ut[:])
                        if z8:
                            zz = tp.tile([P, free], u8, tag="z8")
                            nc.vector.memset(zz[:], 0)
                            st.dma_start(out=z_o[rs, cs], in_=zz[:])
                        else:
                            st.dma_start(out=z_o[rs, cs], in_=vt[:])
                        st.dma_start(out=q_o[rs, cs], in_=qt[:])
                        continue

                    Cc = Ct[:, cs]
                    Wc = Wt[:, cs]
                    Tc = Tt[:, cs]

                    # A = C - DT*q   (fused DVE scalar_tensor_tensor)
                    At = (pp if a_psum else tp).tile([P, free], f32, tag="A")
                    nc.vector.scalar_tensor_tensor(
                        At[:], qt[:], -DT, Cc, Alu.mult, Alu.add
                    )
                    # u_ = (A*u - W*v) + DT*x, written in place over x
                    p1 = tp.tile([P, free], f32, tag="p13")
                    nc.vector.tensor_mul(p1[:], At[:], ut[:])
                    p2 = tp.tile([P, free], f32, tag="p24")
                    nc.vector.tensor_mul(p2[:], Wc, vt[:])
                    u1 = tp.tile([P, free], f32, tag="u1thq")
                    nc.vector.tensor_sub(u1[:], p1[:], p2[:])
                    uo = xt if inplace else iop.tile([P, free], f32, tag="uo")
                    nc.vector.scalar_tensor_tensor(
                        uo[:], xt[:], DT, u1[:], Alu.mult, Alu.add
                    )
                    # thq on POOL before v_ so POOL's W*u, TH+q overlap DVE
                    thq = tp.tile([P, free], f32, tag="u1thq")
                    nc.gpsimd.tensor_tensor(thq[:], Tc, qt[:], Alu.add)
                    p4 = tp.tile([P, free], f32, tag="p24")
                    nc.gpsimd.tensor_tensor(p4[:], Wc, ut[:], Alu.mult)
                    # v_ = A*v + W*u, in place over u
                    p3 = tp.tile([P, free], f32, tag="p13")
                    nc.vector.tensor_mul(p3[:], At[:], vt[:])
                    vo = ut if inplace else iop.tile([P, free], f32, tag="vo")
                    nc.vector.tensor_add(vo[:], p3[:], p4[:])
                    # z = (|u_| > TH + q), f32 in place over v; u8 copy for DMA
                    au = tp.tile([P, free], f32, tag="p13")
                    nc.scalar.activation(au[:], uo[:], Act.Abs)
                    zo = vt if inplace else iop.tile([P, free], f32, tag="zo")
                    nc.vector.tensor_tensor(zo[:], au[:], thq[:], Alu.is_gt)
                    if z8:
                        zz = tp.tile([P, free], u8, tag="z8")
                        nc.scalar.activation(zz[:], zo[:], Act.Copy)
                    else:
                        zz = zo
                    # q_new = 0.9*q + z  (ACT scale, POOL add), in place over q
                    qd = tp.tile([P, free], f32, tag="u1thq")
                    nc.scalar.activation(qd[:], qt[:], Act.Copy, bias=0.0, scale=Q_DECAY)
                    qo = qt if inplace else iop.tile([P, free], f32, tag="qo")
                    nc.gpsimd.tensor_tensor(qo[:], qd[:], zo[:], Alu.add)

                    st.dma_start(out=u_o[rs, cs], in_=uo[:])
                    st.dma_start(out=v_o[rs, cs], in_=vo[:])
                    st.dma_start(out=z_o[rs, cs], in_=zz[:])
                    st.dma_start(out=q_o[rs, cs], in_=qo[:])

    return _install_wait_legalizer(nc)


def host_consts(omegas, bs, threshold):
    """Fold the per-neuron vectors into C/W/TH (fp32, matching jax order)."""
    f = np.float32
    om = np.abs(omegas.astype(np.float32))
    w = (f(DT) * om).astype(np.float32)  # DT*omega
    p = ((f(-1.0) + np.sqrt((f(1.0) - w * w).astype(np.float32))) / f(DT)).astype(
        np.float32
    )
    c1 = (p - np.abs(bs.astype(np.float32))).astype(np.float32)
    c = (f(1.0) + (f(DT) * c1).astype(np.float32)).astype(np.float32)
    th = np.abs(threshold.astype(np.float32))
    d = om.shape[0]
    return c.reshape(1, d), w.reshape(1, d), th.reshape(1, d)


_NC_CACHE = {}


def kernel(x, u, v, q, omegas, bs, threshold):
    global LAST_EXEC_TIME_NS, LAST_RESULTS
    from concourse import bass_utils

    cvec, wvec, tvec = host_consts(omegas, bs, threshold)

    key = "nc"
    if key not in _NC_CACHE:
        _NC_CACHE[key] = build_nc(
            free=2048,
            a_psum=True,
            io_bufs=3,
            tmp_bufs=2,
            inplace=True,
            z8=False,
            store_engine="scalar",
        )
    nc = _NC_CACHE[key]

    x = np.ascontiguousarray(x, dtype=np.float32)
    u = np.ascontiguousarray(u, dtype=np.float32)
    v = np.ascontiguousarray(v, dtype=np.float32)
    q = np.ascontiguousarray(q, dtype=np.float32)

    in_maps = []
    for k in range(N_CORES):
        sl = slice(k * ROWS, (k + 1) * ROWS)
        in_maps.append(
            {
                "x": x[sl],
                "u": u[sl],
                "v": v[sl],
                "q": q[sl],
                "cvec": cvec,
                "wvec": wvec,
                "tvec": tvec,
            }
        )

    trace = bool(int(os.environ.get("BRF_TRACE", "0")))
    res = bass_utils.run_bass_kernel_spmd(
        nc, in_maps, core_ids=list(range(N_CORES)), trace=trace
    )
    LAST_EXEC_TIME_NS = res.exec_time_ns
    LAST_RESULTS = res

    zf = np.concatenate(
        [res.results[k]["z_o"] for k in range(N_CORES)], axis=0
    ).astype(np.float32)
    uf = np.concatenate([res.results[k]["u_o"] for k in range(N_CORES)], axis=0)
    vf = np.concatenate([res.results[k]["v_o"] for k in range(N_CORES)], axis=0)
    qf = np.concatenate([res.results[k]["q_o"] for k in range(N_CORES)], axis=0)
    return (zf, uf, vf, qf)

